# revision 1
# baseline (speedup 1.0000x reference)
"""NCNPredictor v5: bf16 adjacency + scalar_tensor_tensor accumulate.

DVE chain per 128-edge tile (4 wide ops + 2 tiny combines):
  t_all = gi * gj                                  (TT, 3750 bf16)
  u_all = (t_all * 1) * Ycat,  aA = sum            (STT accum: terms 1+3+4)
  (u0 * -1) * t1,              aB = -sum(u0*t1)    (STT accum: term 2)
  (u2 * -1) * t0,              aC = -sum(u2*t0)    (STT accum: term 5)
  acc = aA + aB + aC                               (2 tiny TT adds)
Bias is added host-side in combine.
"""

import sys
from contextlib import ExitStack

import numpy as np

sys.path.insert(0, "/opt/trn_rl_repo")

import concourse.bass as bass
import concourse.tile as tile
from concourse import bacc, mybir
from concourse.bass_utils import run_bass_kernel_spmd

N = 10000
D = 128
E = 8192
NCORES = 8
NCOL = N // NCORES
W3 = 3 * NCOL
E_OWN = E // NCORES
P = 128
ET = E // P
ET_OWN = E_OWN // P
F32 = mybir.dt.float32
BF16 = mybir.dt.bfloat16
I32 = mybir.dt.int32
MUL = mybir.AluOpType.mult
ADD = mybir.AluOpType.add

_CACHE = {}


def _build_nc():
    nc = bacc.Bacc(num_swdge_queues=4)

    acat = nc.declare_dram_parameter("acat", [N, W3], BF16, False)
    xw = nc.declare_dram_parameter("xw", [N, D], F32, False)
    xr = nc.declare_dram_parameter("xr", [N, D], F32, False)
    ycat = nc.declare_dram_parameter("ycat", [P, W3], BF16, False)
    ii = nc.declare_dram_parameter("ii", [E, 1], I32, False)
    jj = nc.declare_dram_parameter("jj", [E, 1], I32, False)
    iown = nc.declare_dram_parameter("iown", [E_OWN, 1], I32, False)
    jown = nc.declare_dram_parameter("jown", [E_OWN, 1], I32, False)

    out_cn = nc.declare_dram_parameter("out_cn", [E, 1], F32, True)
    out_xij = nc.declare_dram_parameter("out_xij", [E_OWN, 1], F32, True)

    with tile.TileContext(nc) as tc, ExitStack() as ctx:
        const = ctx.enter_context(tc.tile_pool(name="const", bufs=1))
        yc = const.tile([P, W3], BF16)
        nc.sync.dma_start(yc[:], ycat[:])

        idxp = ctx.enter_context(tc.tile_pool(name="idxp", bufs=3))
        gat = ctx.enter_context(tc.tile_pool(name="gat", bufs=3))
        msk = ctx.enter_context(tc.tile_pool(name="msk", bufs=2))
        scr = ctx.enter_context(tc.tile_pool(name="scr", bufs=2))
        accp = ctx.enter_context(tc.tile_pool(name="accp", bufs=2))

        for et in range(ET):
            e0 = et * P
            ii_t = idxp.tile([P, 1], I32, name="ii_t")
            nc.sync.dma_start(ii_t[:], ii[e0 : e0 + P, :])
            jj_t = idxp.tile([P, 1], I32, name="jj_t")
            nc.sync.dma_start(jj_t[:], jj[e0 : e0 + P, :])

            gi = gat.tile([P, W3], BF16, name="gi")
            nc.gpsimd.indirect_dma_start(
                out=gi[:], out_offset=None, in_=acat[:],
                in_offset=bass.IndirectOffsetOnAxis(ap=ii_t[:, :1], axis=0),
            )
            gj = gat.tile([P, W3], BF16, name="gj")
            nc.gpsimd.indirect_dma_start(
                out=gj[:], out_offset=None, in_=acat[:],
                in_offset=bass.IndirectOffsetOnAxis(ap=jj_t[:, :1], axis=0),
            )

            t_all = msk.tile([P, W3], BF16, name="t_all")
            nc.vector.tensor_tensor(out=t_all[:], in0=gi[:], in1=gj[:], op=MUL)

            u_all = msk.tile([P, W3], BF16, name="u_all")
            oj = scr.tile([P, NCOL], BF16, name="oj")
            oj2 = scr.tile([P, NCOL], BF16, name="oj2")
            a = [accp.tile([P, 1], F32, name=f"a{s}") for s in range(5)]
            nc.vector.scalar_tensor_tensor(
                out=u_all[:], in0=t_all[:], scalar=1.0, in1=yc[:],
                op0=MUL, op1=MUL, accum_out=a[0][:],
            )
            nc.vector.scalar_tensor_tensor(
                out=oj[:], in0=u_all[:, 0:NCOL], scalar=-1.0,
                in1=t_all[:, NCOL : 2 * NCOL], op0=MUL, op1=MUL,
                accum_out=a[1][:],
            )
            nc.vector.scalar_tensor_tensor(
                out=oj2[:], in0=u_all[:, 2 * NCOL : W3], scalar=-1.0,
                in1=t_all[:, 0:NCOL], op0=MUL, op1=MUL, accum_out=a[2][:],
            )
            nc.vector.tensor_tensor(out=a[3][:], in0=a[0][:], in1=a[1][:], op=ADD)
            nc.vector.tensor_tensor(out=a[4][:], in0=a[3][:], in1=a[2][:], op=ADD)
            nc.sync.dma_start(out_cn[e0 : e0 + P, :], a[4][:])

        for et in range(ET_OWN):
            e0 = et * P
            io_t = idxp.tile([P, 1], I32, name="io_t")
            nc.sync.dma_start(io_t[:], iown[e0 : e0 + P, :])
            jo_t = idxp.tile([P, 1], I32, name="jo_t")
            nc.sync.dma_start(jo_t[:], jown[e0 : e0 + P, :])

            xi_t = gat.tile([P, D], F32, name="xi_t")
            nc.gpsimd.indirect_dma_start(
                out=xi_t[:], out_offset=None, in_=xw[:],
                in_offset=bass.IndirectOffsetOnAxis(ap=io_t[:, :1], axis=0),
            )
            xj_t = gat.tile([P, D], F32, name="xj_t")
            nc.gpsimd.indirect_dma_start(
                out=xj_t[:], out_offset=None, in_=xr[:],
                in_offset=bass.IndirectOffsetOnAxis(ap=jo_t[:, :1], axis=0),
            )
            oxe = scr.tile([P, D], F32, name="oxe")
            oxa = accp.tile([P, 1], F32, name="oxa")
            nc.vector.scalar_tensor_tensor(
                out=oxe[:], in0=xi_t[:], scalar=1.0, in1=xj_t[:],
                op0=MUL, op1=MUL, accum_out=oxa[:],
            )
            nc.sync.dma_start(out_xij[e0 : e0 + P, :], oxa[:])

    return nc


def get_nc():
    if "nc" not in _CACHE:
        nc = _build_nc()
        nc.compile()
        _CACHE["nc"] = nc
    return _CACHE["nc"]


def make_in_maps(x, adj_0_1, adj_1, adj_0_1_2, tar_ei, Wxs, bxs):
    import ml_dtypes

    bf = ml_dtypes.bfloat16
    x32 = np.ascontiguousarray(x, dtype=np.float32)
    wxs = np.asarray(Wxs, dtype=np.float32)
    w0 = wxs[0:D, 0]
    wy = np.concatenate(
        [wxs[D : 2 * D], wxs[2 * D : 3 * D], wxs[3 * D : 4 * D]], axis=1
    )
    y = x32 @ wy
    xwf = np.ascontiguousarray(x32 * w0[None, :])
    ii_all = np.ascontiguousarray(tar_ei[0].astype(np.int32).reshape(E, 1))
    jj_all = np.ascontiguousarray(tar_ei[1].astype(np.int32).reshape(E, 1))

    a01b = adj_0_1.astype(bf)
    a1b = adj_1.astype(bf)
    a012b = adj_0_1_2.astype(bf)
    yb = y.astype(bf)

    in_maps = []
    for c in range(NCORES):
        c0 = c * NCOL
        esl = slice(c * E_OWN, (c + 1) * E_OWN)
        acat = np.empty((N, W3), dtype=bf)
        acat[:, 0:NCOL] = a01b[:, c0 : c0 + NCOL]
        acat[:, NCOL : 2 * NCOL] = a1b[:, c0 : c0 + NCOL]
        acat[:, 2 * NCOL : W3] = a012b[:, c0 : c0 + NCOL]
        ycat = np.empty((P, W3), dtype=bf)
        for k in range(3):
            ycat[:, k * NCOL : (k + 1) * NCOL] = yb[c0 : c0 + NCOL, k][None, :]
        in_maps.append({
            "acat": acat,
            "xw": xwf,
            "xr": x32,
            "ycat": ycat,
            "ii": ii_all,
            "jj": jj_all,
            "iown": np.ascontiguousarray(ii_all[esl]),
            "jown": np.ascontiguousarray(jj_all[esl]),
        })
    return in_maps


def combine_results(results, b):
    out = np.zeros((E, 1), dtype=np.float64)
    for c in range(NCORES):
        out += results[c]["out_cn"].astype(np.float64)
        out[c * E_OWN : (c + 1) * E_OWN] += results[c]["out_xij"].astype(np.float64)
    return (out + b).astype(np.float32)


def kernel(x, adj_0_1, adj_1, adj_0_1_2, tar_ei, Wxs, bxs):
    nc = get_nc()
    in_maps = make_in_maps(x, adj_0_1, adj_1, adj_0_1_2, tar_ei, Wxs, bxs)
    res = run_bass_kernel_spmd(nc, in_maps, list(range(NCORES)))
    b = float(np.asarray(bxs, dtype=np.float32).reshape(-1)[0])
    return combine_results(res.results, b)



# revision 11
# speedup vs baseline: 8.1258x; 8.1258x over previous
"""NCNPredictor v6: fp8 mask tables + transposed gathers + PE matvec reduce.

Formulation: out[e] = xij.w0 + sum_n [cn0*y1 + cn1*y2 + cn2*y3] where
cn0 = t01 & ~t1, cn1 = t1, cn2 = t012 & ~t01 and t_k = A_k[i] & A_k[j].

Per core (column shard of 1280 cols, padded N=10240):
- DRAM table [10000, 3840] fp8: row = [A01|A012|A1] col-shard, values {0, 1.0}.
- dma_gather(transpose=True): per 512-edge block, gi/gj [128, 30, 512] fp8
  (partition p, half-plane d, free 2e+b holds col 2*((d//2)*128+p)+b of matrix d//10).
- DVE: int32-bitcast bitwise ops: t = gi&gj; u0 = t01&t1; u2 = t012&t01;
  m = [t01^u0 | t012^u2] (= cn0, cn2 masks as fp8 {0,1}).
- PE: 30 accumulating matvecs per block: psum[1, 512] += y_col.T @ mask_plane
  (lhsT bf16 [128,1], rhs fp8 stride-2). 3 groups x 5 planes x 2 parities.
- Tiny gather-dependent dummy matmuls keep the PE HAM clock warm.
- xij term: f32 indirect gathers + STT accumulate (as v5), edge-sharded.
Host combines: sum of per-core out_cn + scattered out_xij + bias.
"""

import sys
from contextlib import ExitStack

import numpy as np

sys.path.insert(0, "/opt/trn_rl_repo")

import concourse.bass as bass
import concourse.tile as tile
from concourse import bacc, mybir
from concourse.bass_utils import run_bass_kernel_spmd

N = 10000
D = 128
E = 8192
NCORES = 8
NCOLP = 1280          # padded columns per core
ELEM = 3 * NCOLP      # 3840 bytes per table row
NPAD = NCORES * NCOLP
NI = 512              # edges per block
NBLK = E // NI
E_OWN = E // NCORES
P = 128
ET_OWN = E_OWN // P
F32 = mybir.dt.float32
BF16 = mybir.dt.bfloat16
I32 = mybir.dt.int32
I16 = mybir.dt.int16
FP8 = mybir.dt.float8e4
MUL = mybir.AluOpType.mult
BAND = mybir.AluOpType.bitwise_and
BXOR = mybir.AluOpType.bitwise_xor

_CACHE = {}


def _build_nc(repeat=1):
    nc = bacc.Bacc(num_swdge_queues=4)

    tab = nc.declare_dram_parameter("tab", [N, ELEM], FP8, False)
    idx_i = nc.declare_dram_parameter("idx_i", [128, E // 16], I16, False)
    idx_j = nc.declare_dram_parameter("idx_j", [128, E // 16], I16, False)
    yw = nc.declare_dram_parameter("yw", [128, 90], BF16, False)
    xw = nc.declare_dram_parameter("xw", [N, D], F32, False)
    xr = nc.declare_dram_parameter("xr", [N, D], F32, False)
    idx_io = nc.declare_dram_parameter("idx_io", [128, E_OWN // 16], I16, False)
    idx_jo = nc.declare_dram_parameter("idx_jo", [128, E_OWN // 16], I16, False)

    out_cn = nc.declare_dram_parameter("out_cn", [1, E], F32, True)
    out_xij = nc.declare_dram_parameter("out_xij", [E_OWN, 1], F32, True)

    with tile.TileContext(nc) as tc, ExitStack() as ctx:
        const = ctx.enter_context(tc.tile_pool(name="const", bufs=1))
        ii = const.tile([128, E // 16], I16)
        jj = const.tile([128, E // 16], I16)
        y = const.tile([128, 90], BF16)
        red = const.tile([1, E], F32)
        nc.sync.dma_start(ii[:], idx_i[:])
        nc.sync.dma_start(jj[:], idx_j[:])
        nc.sync.dma_start(y[:], yw[:])

        gpool = ctx.enter_context(tc.tile_pool(name="gpool", bufs=2))
        tpool = ctx.enter_context(tc.tile_pool(name="tpool", bufs=2))
        upool = ctx.enter_context(tc.tile_pool(name="upool", bufs=2))
        ps = ctx.enter_context(tc.tile_pool(name="ps", bufs=2, space="PSUM"))
        wps = ctx.enter_context(tc.tile_pool(name="wps", bufs=1, space="PSUM"))

        # xij: one dma_gather per side for all own edges (queue 3), then
        # per-plane STT accumulate. Edge e -> partition e%128, plane e//128.
        xp = ctx.enter_context(tc.tile_pool(name="xp", bufs=1))
        scr = ctx.enter_context(tc.tile_pool(name="scr", bufs=2))
        accp = ctx.enter_context(tc.tile_pool(name="accp", bufs=2))

        io = xp.tile([128, E_OWN // 16], I16)
        jo = xp.tile([128, E_OWN // 16], I16)
        nc.sync.dma_start(io[:], idx_io[:])
        nc.sync.dma_start(jo[:], idx_jo[:])
        xi_t = xp.tile([P, ET_OWN, D], F32)
        xj_t = xp.tile([P, ET_OWN, D], F32)

        def xij_gathers():
            nc.gpsimd.dma_gather(
                out_ap=xi_t[:], in_ap=xw[:], idxs_ap=io[:], num_idxs=E_OWN,
                num_idxs_reg=E_OWN, elem_size=D, transpose=False, queue_num=3,
            )
            nc.gpsimd.dma_gather(
                out_ap=xj_t[:], in_ap=xr[:], idxs_ap=jo[:], num_idxs=E_OWN,
                num_idxs_reg=E_OWN, elem_size=D, transpose=False, queue_num=3,
            )

        def xij_tile(et):
            oxe = scr.tile([P, D], F32, name="oxe")
            oxa = accp.tile([P, 1], F32, name="oxa")
            nc.vector.scalar_tensor_tensor(
                out=oxe[:], in0=xi_t[:, et, :], scalar=1.0, in1=xj_t[:, et, :],
                op0=MUL, op1=MUL, accum_out=oxa[:],
            )
            nc.sync.dma_start(out_xij[et * P : (et + 1) * P, :], oxa[:])

        W = NI // 16
        warm = wps.tile([1, 16], F32)
        for _rep in range(repeat):
            xij_gathers()
            _blocks(nc, ctx, ii, jj, y, red, gpool, tpool, upool, ps, wps,
                    warm, tab, W, xij_tile)

        nc.sync.dma_start(out_cn[:], red[:])

    return nc


def _blocks(nc, ctx, ii, jj, y, red, gpool, tpool, upool, ps, wps, warm, tab,
            W, xij_tile):
    if True:
        for bl in range(NBLK):
            gi = gpool.tile([128, 30, NI], FP8, name="gi")
            gj = gpool.tile([128, 30, NI], FP8, name="gj")
            nc.gpsimd.dma_gather(
                out_ap=gi[:], in_ap=tab[:], idxs_ap=ii[:, bl * W : (bl + 1) * W],
                num_idxs=NI, num_idxs_reg=NI, elem_size=ELEM, transpose=True,
                queue_num=(2 * bl) % 3,
            )
            # keep-warm matmul tied to gi arrival (prevents PE HAM re-throttle)
            nc.tensor.matmul(
                out=warm[:], lhsT=y[:, 0:1], rhs=gi[:, 0:1, 0:16],
                start=True, stop=True,
            )
            nc.gpsimd.dma_gather(
                out_ap=gj[:], in_ap=tab[:], idxs_ap=jj[:, bl * W : (bl + 1) * W],
                num_idxs=NI, num_idxs_reg=NI, elem_size=ELEM, transpose=True,
                queue_num=(2 * bl + 1) % 3,
            )
            nc.tensor.matmul(
                out=warm[:], lhsT=y[:, 0:1], rhs=gj[:, 0:1, 0:16],
                start=True, stop=True,
            )

            t = tpool.tile([128, 30, NI], FP8, name="t")
            nc.vector.tensor_tensor(
                out=t[:].bitcast(I32), in0=gi[:].bitcast(I32),
                in1=gj[:].bitcast(I32), op=BAND,
            )
            # table order [A01 | A012 | A1]: t01 = 0:10, t012 = 10:20, t1 = 20:30
            u = upool.tile([128, 20, NI], FP8, name="u")
            m = upool.tile([128, 20, NI], FP8, name="m")
            nc.vector.tensor_tensor(
                out=u[:, 0:10, :].bitcast(I32), in0=t[:, 0:10, :].bitcast(I32),
                in1=t[:, 20:30, :].bitcast(I32), op=BAND,
            )
            nc.vector.tensor_tensor(
                out=u[:, 10:20, :].bitcast(I32), in0=t[:, 10:20, :].bitcast(I32),
                in1=t[:, 0:10, :].bitcast(I32), op=BAND,
            )
            nc.vector.tensor_tensor(
                out=m[:].bitcast(I32), in0=t[:, 0:20, :].bitcast(I32),
                in1=u[:].bitcast(I32), op=BXOR,
            )

            acc = ps.tile([1, NI], F32, name="acc")
            k = 0
            for src, base in ((m, 0), (m, 10), (t, 20)):
                for qm in range(5):
                    for b in range(2):
                        rhs = src[:, base + 2 * qm : base + 2 * qm + 2, b::2]
                        nc.tensor.matmul(
                            out=acc[:], lhsT=y[:, k : k + 1], rhs=rhs,
                            start=(k == 0), stop=(k == 29),
                        )
                        k += 1
            nc.scalar.copy(red[:, bl * NI : (bl + 1) * NI], acc[:])

            if bl % 2 == 1 and bl // 2 < ET_OWN:
                xij_tile(bl // 2)


def get_nc():
    if "nc" not in _CACHE:
        nc = _build_nc()
        nc.compile()
        _CACHE["nc"] = nc
    return _CACHE["nc"]


def _mask_table(a, c0):
    """fp8 {0, 1.0} byte table for columns [c0, c0+NCOLP) of binary matrix a."""
    out = np.zeros((N, NCOLP), dtype=np.uint8)
    hi = min(c0 + NCOLP, N)
    if hi > c0:
        out[:, : hi - c0] = np.where(a[:, c0:hi] != 0, np.uint8(0x38), np.uint8(0))
    return out


def make_in_maps(x, adj_0_1, adj_1, adj_0_1_2, tar_ei, Wxs, bxs):
    import ml_dtypes

    bf = ml_dtypes.bfloat16
    x32 = np.ascontiguousarray(x, dtype=np.float32)
    wxs = np.asarray(Wxs, dtype=np.float32)
    w0 = wxs[0:D, 0]
    # y_k = x @ W_k, padded to NPAD with zeros
    ys = np.zeros((3, NPAD), dtype=np.float32)
    for k in range(3):
        ys[k, :N] = x32 @ wxs[(k + 1) * D : (k + 2) * D, 0]
    y1, y2, y3 = ys

    xwf = np.ascontiguousarray(x32 * w0[None, :])
    ii_all = np.asarray(tar_ei[0], dtype=np.int64)
    jj_all = np.asarray(tar_ei[1], dtype=np.int64)

    # wrapped int16 idx tiles: edge e=bl*NI+el at [el%16, bl*W+el//16],
    # replicated across the 8 gpsimd-core partition bands
    def wrap(idx):
        wt = np.zeros((16, E // 16), np.int16)
        el = np.arange(E) % NI
        bl = np.arange(E) // NI
        wt[el % 16, bl * (NI // 16) + el // 16] = idx.astype(np.int16)
        return np.tile(wt, (8, 1))

    ii_w = wrap(ii_all)
    jj_w = wrap(jj_all)

    # xij idx: non-transpose gather, edges wrapped [e%16, e//16], replicated
    def wrap_own(idx):
        wt = np.zeros((16, E_OWN // 16), np.int16)
        el = np.arange(E_OWN)
        wt[el % 16, el // 16] = idx.astype(np.int16)
        return np.tile(wt, (8, 1))

    a01 = np.asarray(adj_0_1)
    a1 = np.asarray(adj_1)
    a012 = np.asarray(adj_0_1_2)

    p_arr = np.arange(128)
    in_maps = []
    for c in range(NCORES):
        c0 = c * NCOLP
        # table: [A01 | A012 | A1] column shard as fp8 {0,1} bytes
        tab = np.empty((N, ELEM), dtype=np.uint8)
        tab[:, 0:NCOLP] = _mask_table(a01, c0)
        tab[:, NCOLP : 2 * NCOLP] = _mask_table(a012, c0)
        tab[:, 2 * NCOLP : 3 * NCOLP] = _mask_table(a1, c0)

        # y weights [128, 90]: groups (m0->y1, m2->y3, t1->y2), cols (g*10+qm*2+b)
        ywc = np.zeros((128, 90), dtype=bf)
        for g, yg in enumerate((y1, y3, y2)):
            for qm in range(5):
                for b in range(2):
                    cols = c0 + 2 * (qm * 128 + p_arr) + b
                    ywc[:, g * 10 + qm * 2 + b] = yg[cols].astype(bf)

        esl = slice(c * E_OWN, (c + 1) * E_OWN)
        in_maps.append({
            "tab": tab.view(ml_dtypes.float8_e4m3),
            "idx_i": ii_w,
            "idx_j": jj_w,
            "yw": ywc,
            "xw": xwf,
            "xr": x32,
            "idx_io": wrap_own(ii_all[esl]),
            "idx_jo": wrap_own(jj_all[esl]),
        })
    return in_maps


def combine_results(results, b):
    out = np.zeros((E, 1), dtype=np.float64)
    for c in range(NCORES):
        out[:, 0] += results[c]["out_cn"].reshape(E).astype(np.float64)
        out[c * E_OWN : (c + 1) * E_OWN] += results[c]["out_xij"].astype(np.float64)
    return (out + b).astype(np.float32)


def kernel(x, adj_0_1, adj_1, adj_0_1_2, tar_ei, Wxs, bxs):
    nc = get_nc()
    in_maps = make_in_maps(x, adj_0_1, adj_1, adj_0_1_2, tar_ei, Wxs, bxs)
    res = run_bass_kernel_spmd(nc, in_maps, list(range(NCORES)))
    b = float(np.asarray(bxs, dtype=np.float32).reshape(-1)[0])
    return combine_results(res.results, b)
